# revision 22
# baseline (speedup 1.0000x reference)
"""Trainium2 Bass kernel for PVT-style spatial-reduction attention.

Reference computation (per batch):
  x_ds = x[:, ::4, ::4]                                  # nearest downsample 192->48
  q    = q_w @ x_ds + q_b                                # 1x1 conv
  d1   = relu(bn1(dwconv2x2_s2_p1(x_ds)))                # 48 -> 25
  kv1  = bn2(sr2_w @ d1)
  kv2  = dwconv3x3_s1_p1(kv1) + lc_b + kv1
  k,v  = split(kv_w @ kv2 + kv_b)
  out  = softmax(q'k/8) @ v  -> reshape [C,48,48] -> nearest upsample x4

Sharding: 8 cores = 4 batches x 2 head-groups (4 heads / 256 ch each).
Host shard-prep stages the strided 48x48 view of x per batch (pure index
selection); each core computes its [256,192,192] output slab entirely on
device, including the 4x4 nearest-neighbour replication, and stores it
with 128-partition fully-contiguous DMAs on both HWDGE rings.
"""

import sys

for _p in ("/root/.axon_site/_ro/trn_rl_repo", "/opt/trn_rl_repo"):
    if _p in sys.path:
        sys.path.remove(_p)
    sys.path.insert(0, _p)

import numpy as np


def _ensure_ntff_hook_module():
    """Provide antenv.axon_hooks (NTFF profile hook registry) if the
    resolved antenv package lacks it — needed for trace=True profiling."""
    try:
        import antenv.axon_hooks  # noqa: F401

        return
    except ImportError:
        pass
    try:
        import types

        import antenv

        mod = types.ModuleType("antenv.axon_hooks")
        mod._HOOK = None

        def set_axon_ntff_profile_hook(hook):
            mod._HOOK = hook

        def get_axon_ntff_profile_hook():
            if mod._HOOK is None:
                try:
                    if "/root/.axon_site" not in sys.path:
                        sys.path.append("/root/.axon_site")
                    from trn_agent_boot.trn_boot import (
                        _ntff_profile_via_ctypes,
                    )

                    mod._HOOK = _ntff_profile_via_ctypes(
                        "/opt/axon/libaxon_pjrt.so"
                    )
                except Exception:
                    mod._HOOK = None
            return mod._HOOK

        mod.set_axon_ntff_profile_hook = set_axon_ntff_profile_hook
        mod.get_axon_ntff_profile_hook = get_axon_ntff_profile_hook
        antenv.axon_hooks = mod
        sys.modules["antenv.axon_hooks"] = mod
    except Exception:
        pass


_ensure_ntff_hook_module()

import concourse.bass as bass
import concourse.tile as tile
from concourse import bacc
from concourse import mybir
from concourse.bass_utils import run_bass_kernel_spmd

F32 = mybir.dt.float32
F32R = mybir.dt.float32r
BF16 = mybir.dt.bfloat16
ALU = mybir.AluOpType
ACTF = mybir.ActivationFunctionType

# Problem constants (hardcoded per contract).
C = 512          # channels
H0 = W0 = 192    # full spatial
HD = WD = 48     # downsampled spatial
N = HD * WD      # 2304 queries
HS = WS = 25     # spatially-reduced size after 2x2/s2/p1 dwconv
M = HS * WS      # 625 keys
HPC = 4          # heads per core
CQ = 256         # q/k/v channels per core
NCORES = 8
BN_EPS = 1e-5
SCALE = 0.125    # hd ** -0.5 = 64 ** -0.5

# n-chunks over the 2304 query positions; each is a multiple of 48 so a
# chunk maps to whole rows of the 48x48 grid, and <=512 so QK / the
# softmax row-sum fit one PSUM bank.
NCH = [(0, 480), (480, 480), (960, 480), (1440, 480), (1920, 384)]
# m-tiles over the 625 key positions (output-partition tiles)
MTS = [(0, 128), (128, 128), (256, 128), (384, 128), (512, 113)]
# free-dim split of the padded 626 kv free dim (fp32r needs even counts)
MP = 626
MFREE = [(0, 512), (512, 114)]


def _build_nc():
    nc = bacc.Bacc("TRN2", target_bir_lowering=False, debug=False)

    xds = nc.dram_tensor("xds", [128, 4, HD, WD], BF16, kind="ExternalInput").ap()
    qwT = nc.dram_tensor("qwT", [128, 4, CQ], BF16, kind="ExternalInput").ap()
    qb = nc.dram_tensor("qb", [128, 2], F32, kind="ExternalInput").ap()
    sr2T = nc.dram_tensor("sr2T", [128, 4, C], BF16, kind="ExternalInput").ap()
    kvkT = nc.dram_tensor("kvkT", [128, 4, CQ], BF16, kind="ExternalInput").ap()
    kvvT = nc.dram_tensor("kvvT", [128, 4, CQ], BF16, kind="ExternalInput").ap()
    kvbk = nc.dram_tensor("kvbk", [128, 2], F32, kind="ExternalInput").ap()
    kvbv = nc.dram_tensor("kvbv", [128, CQ], F32, kind="ExternalInput").ap()
    vecs = nc.dram_tensor("vecs", [128, 4, 18], F32, kind="ExternalInput").ap()
    # f-major output layout: out[f, c, a, w] holds full-res row 4a+f.
    # Every store is then fully contiguous per partition; the host gather
    # interleaves the four row-replicas while assembling the slab.
    out_d = nc.dram_tensor("out", [4, CQ, HD, W0], F32, kind="ExternalOutput").ap()

    with tile.TileContext(nc) as tc:
        with nc.allow_low_precision(
            reason="float32r is fp32-width; matmul accumulation stays fp32"
        ):
            _body(tc, xds, qwT, qb, sr2T, kvkT, kvvT, kvbk, kvbv, vecs, out_d)
    nc.compile()
    return nc


def _body(tc, xds, qwT, qb, sr2T, kvkT, kvvT, kvbk, kvbv, vecs, out_d):
    nc = tc.nc
    from contextlib import ExitStack

    with ExitStack() as ctx:
        # ---- constants / weights: staged on the Activation HWDGE ring so
        # the x chunks own the SP ring ----
        consts = ctx.enter_context(tc.tile_pool(name="consts", bufs=1))
        qwT_sb = consts.tile([128, 4, CQ], BF16)
        nc.scalar.dma_start(out=qwT_sb, in_=qwT)
        qb_sb = consts.tile([128, 2], F32)
        nc.scalar.dma_start(out=qb_sb, in_=qb)
        vecs_sb = consts.tile([128, 4, 18], F32)
        nc.scalar.dma_start(out=vecs_sb, in_=vecs)
        sr2T_sb = consts.tile([128, 4, C], BF16)
        nc.scalar.dma_start(out=sr2T_sb, in_=sr2T)
        kvkT_sb = consts.tile([128, 4, CQ], BF16)
        nc.scalar.dma_start(out=kvkT_sb, in_=kvkT)
        kvvT_sb = consts.tile([128, 4, CQ], BF16)
        nc.scalar.dma_start(out=kvvT_sb, in_=kvvT)
        kvbk_sb = consts.tile([128, 2], F32)
        nc.scalar.dma_start(out=kvbk_sb, in_=kvbk)
        kvbv_sb = consts.tile([128, CQ], F32)
        nc.scalar.dma_start(out=kvbv_sb, in_=kvbv)
        zsmall = consts.tile([128, 1], F32)
        nc.vector.memset(zsmall, 0.0)
        osmall = consts.tile([128, 1], F32)
        nc.vector.memset(osmall, 1.0)
        ones_sb = consts.tile([65, 64], F32R)
        nc.vector.tensor_copy(
            out=ones_sb, in_=osmall[0:65, :].to_broadcast([65, 64])
        )

        persist = ctx.enter_context(tc.tile_pool(name="persist", bufs=1))
        q_sb = persist.tile([128, 2, N], BF16)
        k_loc = persist.tile([128, 2, M], BF16)
        vT_sb = persist.tile([128, 5, HPC, 65], BF16)

        with ExitStack() as pctx:
            xp = pctx.enter_context(tc.tile_pool(name="xp", bufs=1))
            mid = pctx.enter_context(tc.tile_pool(name="mid", bufs=1))
            psB = pctx.enter_context(
                tc.tile_pool(name="psB", bufs=2, space="PSUM")
            )
            psV = pctx.enter_context(
                tc.tile_pool(name="psV", bufs=2, space="PSUM")
            )

            # ---- Phase A: load pre-strided x_ds in per-cc chunks ----
            x_ds = xp.tile([128, 4, HD, WD], BF16)
            for cc in range(4):
                nc.sync.dma_start(out=x_ds[:, cc], in_=xds[:, cc])
            x_flat = x_ds.rearrange("p c h w -> p c (h w)")

            # ---- Phase W: PE warmup during the input load (HAM clock
            # gate releases after ~3.4us of sustained matmul activity) ----
            warm_src = consts.tile([128, 512], F32R)
            nc.vector.tensor_copy(
                out=warm_src, in_=zsmall.to_broadcast([128, 512])
            )
            def pe_warm(nwarm, tag, sink):
                with tc.tile_pool(name=tag, bufs=1, space="PSUM") as psW:
                    wps = psW.tile([128, 512], F32, tag=tag)
                    for i in range(nwarm):
                        nc.tensor.matmul(
                            wps,
                            lhsT=warm_src[:, 0:128],
                            rhs=warm_src,
                            start=(i == 0),
                            stop=(i == nwarm - 1),
                        )
                    # consume so the chain isn't dead
                    nc.vector.tensor_copy(
                        out=warm_src[:, sink:sink + 2], in_=wps[:, 0:2]
                    )

            pe_warm(30, "psW", 0)

            # ---- Phase C: depthwise 2x2 stride-2 pad-1 conv + BN1 + ReLU
            # taps split across Vector and GpSimd ----
            d1 = mid.tile([128, 4, MP], BF16)
            nc.vector.tensor_copy(
                out=d1, in_=zsmall.to_broadcast([128, 4, MP])
            )
            d1s = d1[:, :, 0:M].rearrange("p c (h w) -> p c h w", h=HS)
            xv = x_ds.rearrange("p c (h t) (w u) -> p c h t w u", t=2, u=2)
            for cc in range(4):
                eng = nc.vector
                for ki in (0, 1):
                    ro = slice(1, 25) if ki == 0 else slice(0, 24)
                    for kj in (0, 1):
                        co = slice(1, 25) if kj == 0 else slice(0, 24)
                        src = xv[:, cc, :, 1 - ki, :, 1 - kj]
                        dst = d1s[:, cc, ro, co]
                        eng.scalar_tensor_tensor(
                            out=dst,
                            in0=src,
                            scalar=vecs_sb[:, cc, ki * 2 + kj:ki * 2 + kj + 1],
                            in1=dst,
                            op0=ALU.mult,
                            op1=ALU.add,
                        )
            for cc in range(4):
                nc.scalar.activation(
                    out=d1s[:, cc],
                    in_=d1s[:, cc],
                    func=ACTF.Relu,
                    bias=vecs_sb[:, cc, 5:6],
                    scale=vecs_sb[:, cc, 4:5],
                )

            # ---- Phase B: q projection (256 out-ch, K=512) ----
            for mt in range(2):
                for (n0, nn) in NCH:
                    ps = psB.tile([128, MP], F32, tag="psB")
                    for cc in range(4):
                        nc.tensor.matmul(
                            ps[:, 0:nn],
                            lhsT=qwT_sb[:, cc, mt * 128:(mt + 1) * 128],
                            rhs=x_flat[:, cc, n0:n0 + nn],
                            start=(cc == 0),
                            stop=(cc == 3),
                        )
                    nc.scalar.activation(
                        out=q_sb[:, mt, n0:n0 + nn],
                        in_=ps[:, 0:nn],
                        func=ACTF.Identity,
                        bias=qb_sb[:, mt:mt + 1],
                        scale=1.0,
                    )

            # ---- Phase D: sr2 1x1 conv + BN2 ----
            d1f = d1
            kv1 = mid.tile([128, 4, HS, WS], BF16)
            kv1f = kv1.rearrange("p c h w -> p c (h w)")
            for mt in range(4):
                ps = psB.tile([128, MP], F32, tag="psB")
                for (f0, ff) in MFREE:
                    for cc in range(4):
                        nc.tensor.matmul(
                            ps[:, f0:f0 + ff],
                            lhsT=sr2T_sb[:, cc, mt * 128:(mt + 1) * 128],
                            rhs=d1f[:, cc, f0:f0 + ff],
                            start=(cc == 0),
                            stop=(cc == 3),
                        )
                nc.scalar.activation(
                    out=kv1f[:, mt],
                    in_=ps[:, 0:M],
                    func=ACTF.Identity,
                    bias=vecs_sb[:, mt, 7:8],
                    scale=vecs_sb[:, mt, 6:7],
                )

            # keep the PE's HAM activity window busy across the E-phase
            # DVE wall so F and G start at full clock
            pe_warm(56, "psW2", 2)

            # ---- Phase E: depthwise 3x3 pad-1 conv + lc_b + residual ----
            kv2 = mid.tile([128, 4, MP], BF16)
            nc.vector.tensor_copy(
                out=kv2[:, :, M:MP], in_=zsmall.to_broadcast([128, 4, MP - M])
            )
            kv2s = kv2[:, :, 0:M].rearrange("p c (h w) -> p c h w", h=HS)
            for cc in range(4):
                # center tap: kv2 = (w11 + 1) * kv1 + lc_b  (residual folded)
                nc.scalar.activation(
                    out=kv2s[:, cc],
                    in_=kv1[:, cc],
                    func=ACTF.Identity,
                    bias=vecs_sb[:, cc, 17:18],
                    scale=vecs_sb[:, cc, 12:13],
                )
                eng = nc.vector
                for ki in range(3):
                    for kj in range(3):
                        if ki == 1 and kj == 1:
                            continue
                        di, dj = ki - 1, kj - 1
                        a0, a1 = max(0, -di), 25 - max(0, di)
                        b0, b1 = max(0, -dj), 25 - max(0, dj)
                        src = kv1[:, cc, a0 + di:a1 + di, b0 + dj:b1 + dj]
                        dst = kv2s[:, cc, a0:a1, b0:b1]
                        s = 8 + ki * 3 + kj
                        eng.scalar_tensor_tensor(
                            out=dst,
                            in0=src,
                            scalar=vecs_sb[:, cc, s:s + 1],
                            in1=dst,
                            op0=ALU.mult,
                            op1=ALU.add,
                        )

            # ---- Phase F: k and v projections ----
            kv2f = kv2
            for kt in range(2):
                ps = psB.tile([128, MP], F32, tag="psB")
                for (f0, ff) in MFREE:
                    for cc in range(4):
                        nc.tensor.matmul(
                            ps[:, f0:f0 + ff],
                            lhsT=kvkT_sb[:, cc, kt * 128:(kt + 1) * 128],
                            rhs=kv2f[:, cc, f0:f0 + ff],
                            start=(cc == 0),
                            stop=(cc == 3),
                        )
                nc.scalar.activation(
                    out=k_loc[:, kt],
                    in_=ps[:, 0:M],
                    func=ACTF.Identity,
                    bias=kvbk_sb[:, kt:kt + 1],
                    scale=1.0,
                )

            # v, produced directly transposed: vT[m, d] (+ ones column)
            nc.vector.tensor_copy(
                out=vT_sb[:, :, :, 64], in_=osmall.to_broadcast([128, 5, HPC])
            )
            kvbv_h = kvbv_sb.rearrange("p (h d) -> p h d", h=HPC)
            for mi, (m0, msz) in enumerate(MTS):
                ps = psV.tile([128, CQ], F32, tag="psV")
                for cc in range(4):
                    nc.tensor.matmul(
                        ps[:msz],
                        lhsT=kv2f[:, cc, m0:m0 + msz],
                        rhs=kvvT_sb[:, cc],
                        start=(cc == 0),
                        stop=(cc == 3),
                    )
                nc.vector.tensor_tensor(
                    out=vT_sb[:msz, mi, :, 0:64],
                    in0=ps[:msz].rearrange("p (h d) -> p h d", h=HPC),
                    in1=kvbv_h[:msz],
                    op=ALU.add,
                )

        # ---- Phase G: attention, normalize, W-replicate, store ----
        # Heads are processed in pairs so each store covers 128 contiguous
        # output channels. The x4 column replication is fused into the
        # normalize op; the x4 row replication is four DMAs of the same
        # tile into the f-major output layout, each fully contiguous per
        # partition (128 partitions -> all 16 SDMA engines, ~HBM line
        # rate), alternating HWDGE rings.
        with ExitStack() as gctx:
            oa_pool = gctx.enter_context(
                tc.tile_pool(name="oa", bufs=3, space="PSUM")
            )
            qk_pool = gctx.enter_context(
                tc.tile_pool(name="qk", bufs=5, space="PSUM")
            )
            e_pool = gctx.enter_context(tc.tile_pool(name="es", bufs=6))
            nw_pool = gctx.enter_context(tc.tile_pool(name="nw", bufs=4))
            r_pool = gctx.enter_context(tc.tile_pool(name="rp", bufs=2))

            # The reciprocal / PE-broadcast / normalize of head-chunk j is
            # emitted after head-chunk j+1's matmuls: the tiny `bc`
            # broadcast matmul then reaches the PE with its DVE deps long
            # resolved, so the PE stream stays dense (HAM stays warm).
            pending = []   # (oa, nw, hp, nn)
            ready = {}     # id(nw) -> [norms_done, store_args]
            si = 0

            def emit_norm():
                oa, nw, hp, nn = pending.pop(0)
                rp = r_pool.tile([65, 512], F32, tag="rp")
                nc.vector.reciprocal_approx_fast(
                    out=rp[:, 0:nn], in_=oa[:, 0:nn]
                )
                rt = e_pool.tile([128, 512], F32R, tag="es")
                nc.vector.tensor_copy(
                    out=rt[64:65, 0:nn], in_=rp[64:65, 0:nn]
                )
                bc = qk_pool.tile([64, 512], F32, tag="qk")
                nc.tensor.matmul(
                    bc[:, 0:nn],
                    lhsT=ones_sb[64:65, :],
                    rhs=rt[64:65, 0:nn],
                    start=True,
                    stop=True,
                )
                # TT can read only one PSUM operand: stage 1/rowsum in
                # SBUF via ACT (ScalarE has slack here)
                bcs = e_pool.tile([128, 512], F32R, tag="es")
                nc.vector.tensor_copy(out=bcs[0:64, 0:nn], in_=bc[:, 0:nn])
                # fused normalize + x4 column replication
                nwr = nw[hp:hp + 64].rearrange("p a (w s) -> p (a w) s", s=4)
                src0 = oa[0:64, 0:nn].unsqueeze(2).broadcast_to([64, nn, 4])
                src1 = bcs[0:64, 0:nn].unsqueeze(2).broadcast_to([64, nn, 4])
                # (GpSimd cannot read PSUM: normalize stays on DVE)
                nc.vector.tensor_tensor(
                    out=nwr, in0=src0, in1=src1, op=ALU.mult
                )
                st = ready[id(nw)]
                st[0] += 1
                if st[0] == 2:
                    emit_store(nw, *st[1])

            def emit_store(nw, pp, a0, nr):
                nonlocal si
                # x4 row replication: store the tile into all four
                # f-planes of the f-major output layout
                engs = [nc.sync, nc.scalar, nc.gpsimd]
                for f in range(4):
                    eng = engs[(si + f) % 3]
                    eng.dma_start(
                        out=out_d[f, pp * 128:(pp + 1) * 128, a0:a0 + nr, :],
                        in_=nw,
                    )
                si += 1

            for pp in range(2):          # head pair: heads (2pp, 2pp+1)
                for (n0, nn) in NCH:
                    nr = nn // WD        # whole 48-wide rows in this chunk
                    a0 = n0 // WD
                    nw = nw_pool.tile([128, nr, W0], F32, tag="nw")
                    ready[id(nw)] = [0, (pp, a0, nr)]
                    for hh in range(2):
                        h = pp * 2 + hh
                        hp = hh * 64
                        oa = oa_pool.tile([65, 512], F32, tag="oa")
                        # all QK matmuls first (exp trails on ScalarE),
                        # then all AV matmuls: keeps the PE stream dense
                        es = []
                        for mi, (m0, msz) in enumerate(MTS):
                            qs = qk_pool.tile([128, 512], F32, tag="qk")
                            nc.tensor.matmul(
                                qs[:msz, 0:nn],
                                lhsT=k_loc[hp:hp + 64, pp, m0:m0 + msz],
                                rhs=q_sb[hp:hp + 64, pp, n0:n0 + nn],
                                start=True,
                                stop=True,
                            )
                            e = e_pool.tile([128, 512], BF16, tag="es")
                            nc.scalar.activation(
                                out=e[:msz, 0:nn],
                                in_=qs[:msz, 0:nn],
                                func=ACTF.Exp,
                                scale=SCALE,
                            )
                            es.append(e)
                        for mi, (m0, msz) in enumerate(MTS):
                            nc.tensor.matmul(
                                oa[:, 0:nn],
                                lhsT=vT_sb[:msz, mi, h, :],
                                rhs=es[mi][:msz, 0:nn],
                                start=(mi == 0),
                                stop=(mi == 4),
                            )
                        pending.append((oa, nw, hp, nn))
                        while len(pending) > 1:
                            emit_norm()
            while pending:
                emit_norm()


_NC_CACHE = None


def _get_nc():
    global _NC_CACHE
    if _NC_CACHE is None:
        _NC_CACHE = _build_nc()
    return _NC_CACHE


def _prep_in_maps(inputs):
    x = np.asarray(inputs["x"], dtype=np.float32)
    q_w = np.asarray(inputs["q_w"], dtype=np.float32)
    q_b = np.asarray(inputs["q_b"], dtype=np.float32)
    kv_w = np.asarray(inputs["kv_w"], dtype=np.float32)
    kv_b = np.asarray(inputs["kv_b"], dtype=np.float32)
    sr1_w = np.asarray(inputs["sr1_w"], dtype=np.float32)
    bn1 = [np.asarray(inputs[f"bn1_{t}"], dtype=np.float32) for t in "gbmv"]
    sr2_w = np.asarray(inputs["sr2_w"], dtype=np.float32)
    bn2 = [np.asarray(inputs[f"bn2_{t}"], dtype=np.float32) for t in "gbmv"]
    lc_w = np.asarray(inputs["lc_w"], dtype=np.float32)
    lc_b = np.asarray(inputs["lc_b"], dtype=np.float32)

    def chan_layout(vec_2d):
        # [C, k] -> [128, 4, k] with channel = cc*128 + p
        k = vec_2d.shape[1]
        return np.ascontiguousarray(
            vec_2d.reshape(4, 128, k).transpose(1, 0, 2)
        )

    s1 = bn1[0] / np.sqrt(bn1[3] + BN_EPS)
    b1 = bn1[1] - bn1[2] * s1
    s2 = bn2[0] / np.sqrt(bn2[3] + BN_EPS)
    b2 = bn2[1] - bn2[2] * s2
    lc = lc_w.reshape(C, 9).copy()
    lc[:, 4] += 1.0  # fold residual into center tap
    vecs = np.zeros((C, 18), np.float32)
    vecs[:, 0:4] = sr1_w.reshape(C, 4)
    vecs[:, 4] = s1
    vecs[:, 5] = b1
    vecs[:, 6] = s2
    vecs[:, 7] = b2
    vecs[:, 8:17] = lc
    vecs[:, 17] = lc_b
    vecs_l = chan_layout(vecs)
    import ml_dtypes

    bf16 = ml_dtypes.bfloat16
    sr2T_l = chan_layout(sr2_w.T.copy()).astype(bf16)

    in_maps = []
    for b in range(4):
        # shard-prep: the strided nearest-downsample view of this batch,
        # laid out [partition, cc, h, w] with channel = cc*128 + p
        xb_ds = np.ascontiguousarray(
            x[b, :, ::4, ::4].reshape(4, 128, HD, WD).transpose(1, 0, 2, 3)
        ).astype(bf16)
        for hg in range(2):
            sl = slice(hg * CQ, (hg + 1) * CQ)
            qwT = chan_layout(q_w[sl].T.copy()).astype(bf16)
            qb_l = np.ascontiguousarray(q_b[sl].reshape(2, 128).T)
            kvkT = chan_layout(kv_w[sl].T.copy()).astype(bf16)
            kvvT = chan_layout(
                kv_w[C + hg * CQ:C + (hg + 1) * CQ].T.copy()
            ).astype(bf16)
            kvbk = np.ascontiguousarray(kv_b[sl].reshape(2, 128).T)
            kvbv = np.ascontiguousarray(
                np.broadcast_to(
                    kv_b[C + hg * CQ:C + (hg + 1) * CQ], (128, CQ)
                ).copy()
            )
            in_maps.append(
                {
                    "xds": xb_ds,
                    "qwT": qwT,
                    "qb": qb_l,
                    "sr2T": sr2T_l,
                    "kvkT": kvkT,
                    "kvvT": kvvT,
                    "kvbk": kvbk,
                    "kvbv": kvbv,
                    "vecs": vecs_l,
                }
            )
    return in_maps


def run(inputs, trace=False, **spmd_kwargs):
    """Run the SPMD kernel; returns (output, BassKernelResults)."""
    nc = _get_nc()
    in_maps = _prep_in_maps(inputs)
    res = run_bass_kernel_spmd(
        nc, in_maps, core_ids=list(range(NCORES)), trace=trace, **spmd_kwargs
    )
    out = np.empty((4, C, H0, W0), np.float32)
    i = 0
    for b in range(4):
        for hg in range(2):
            # device layout [f, c, a, w] -> slab [c, 4a+f, w]
            dev = res.results[i]["out"]
            out[b, hg * CQ:(hg + 1) * CQ] = (
                dev.transpose(1, 2, 0, 3).reshape(CQ, H0, W0)
            )
            i += 1
    return out, res


def kernel(**inputs):
    out, _ = run(inputs, trace=False)
    return out


# revision 23
# speedup vs baseline: 1.0873x; 1.0873x over previous
"""Trainium2 Bass kernel for PVT-style spatial-reduction attention.

Reference computation (per batch):
  x_ds = x[:, ::4, ::4]                                  # nearest downsample 192->48
  q    = q_w @ x_ds + q_b                                # 1x1 conv
  d1   = relu(bn1(dwconv2x2_s2_p1(x_ds)))                # 48 -> 25
  kv1  = bn2(sr2_w @ d1)
  kv2  = dwconv3x3_s1_p1(kv1) + lc_b + kv1
  k,v  = split(kv_w @ kv2 + kv_b)
  out  = softmax(q'k/8) @ v  -> reshape [C,48,48] -> nearest upsample x4

Sharding: 8 cores = 4 batches x 2 head-groups (4 heads / 256 ch each).
Host shard-prep stages the strided 48x48 view of x per batch (pure index
selection); each core computes its [256,192,192] output slab entirely on
device, including the 4x4 nearest-neighbour replication, and stores it
with 128-partition fully-contiguous DMAs on both HWDGE rings.
"""

import sys

for _p in ("/root/.axon_site/_ro/trn_rl_repo", "/opt/trn_rl_repo"):
    if _p in sys.path:
        sys.path.remove(_p)
    sys.path.insert(0, _p)

import numpy as np


def _ensure_ntff_hook_module():
    """Provide antenv.axon_hooks (NTFF profile hook registry) if the
    resolved antenv package lacks it — needed for trace=True profiling."""
    try:
        import antenv.axon_hooks  # noqa: F401

        return
    except ImportError:
        pass
    try:
        import types

        import antenv

        mod = types.ModuleType("antenv.axon_hooks")
        mod._HOOK = None

        def set_axon_ntff_profile_hook(hook):
            mod._HOOK = hook

        def get_axon_ntff_profile_hook():
            if mod._HOOK is None:
                try:
                    if "/root/.axon_site" not in sys.path:
                        sys.path.append("/root/.axon_site")
                    from trn_agent_boot.trn_boot import (
                        _ntff_profile_via_ctypes,
                    )

                    mod._HOOK = _ntff_profile_via_ctypes(
                        "/opt/axon/libaxon_pjrt.so"
                    )
                except Exception:
                    mod._HOOK = None
            return mod._HOOK

        mod.set_axon_ntff_profile_hook = set_axon_ntff_profile_hook
        mod.get_axon_ntff_profile_hook = get_axon_ntff_profile_hook
        antenv.axon_hooks = mod
        sys.modules["antenv.axon_hooks"] = mod
    except Exception:
        pass


_ensure_ntff_hook_module()

import concourse.bass as bass
import concourse.tile as tile
from concourse import bacc
from concourse import mybir
from concourse.bass_utils import run_bass_kernel_spmd

F32 = mybir.dt.float32
F32R = mybir.dt.float32r
BF16 = mybir.dt.bfloat16
ALU = mybir.AluOpType
ACTF = mybir.ActivationFunctionType

# Problem constants (hardcoded per contract).
C = 512          # channels
H0 = W0 = 192    # full spatial
HD = WD = 48     # downsampled spatial
N = HD * WD      # 2304 queries
HS = WS = 25     # spatially-reduced size after 2x2/s2/p1 dwconv
M = HS * WS      # 625 keys
HPC = 4          # heads per core
CQ = 256         # q/k/v channels per core
NCORES = 8
BN_EPS = 1e-5
SCALE = 0.125    # hd ** -0.5 = 64 ** -0.5

# n-chunks over the 2304 query positions; each is a multiple of 48 so a
# chunk maps to whole rows of the 48x48 grid, and <=512 so QK / the
# softmax row-sum fit one PSUM bank.
NCH = [(0, 480), (480, 480), (960, 480), (1440, 480), (1920, 384)]
# m-tiles over the 625 key positions (output-partition tiles)
MTS = [(0, 128), (128, 128), (256, 128), (384, 128), (512, 113)]
# free-dim split of the padded 626 kv free dim (fp32r needs even counts)
MP = 626
MFREE = [(0, 512), (512, 114)]


def _build_nc():
    nc = bacc.Bacc("TRN2", target_bir_lowering=False, debug=False)

    xds = nc.dram_tensor("xds", [128, 4, HD, WD], BF16, kind="ExternalInput").ap()
    qwT = nc.dram_tensor("qwT", [128, 4, CQ], BF16, kind="ExternalInput").ap()
    qb = nc.dram_tensor("qb", [128, 2], F32, kind="ExternalInput").ap()
    sr2T = nc.dram_tensor("sr2T", [128, 4, C], BF16, kind="ExternalInput").ap()
    kvkT = nc.dram_tensor("kvkT", [128, 4, CQ], BF16, kind="ExternalInput").ap()
    kvvT = nc.dram_tensor("kvvT", [128, 4, CQ], BF16, kind="ExternalInput").ap()
    kvbk = nc.dram_tensor("kvbk", [128, 2], F32, kind="ExternalInput").ap()
    kvbv = nc.dram_tensor("kvbv", [128, CQ], F32, kind="ExternalInput").ap()
    vecs = nc.dram_tensor("vecs", [128, 4, 18], F32, kind="ExternalInput").ap()
    # f-major output layout: out[f, c, a, w] holds full-res row 4a+f.
    # Every store is then fully contiguous per partition; the host gather
    # interleaves the four row-replicas while assembling the slab.
    out_d = nc.dram_tensor("out", [4, CQ, HD, W0], F32, kind="ExternalOutput").ap()

    with tile.TileContext(nc) as tc:
        with nc.allow_low_precision(
            reason="float32r is fp32-width; matmul accumulation stays fp32"
        ):
            _body(tc, xds, qwT, qb, sr2T, kvkT, kvvT, kvbk, kvbv, vecs, out_d)
    nc.compile()
    return nc


def _body(tc, xds, qwT, qb, sr2T, kvkT, kvvT, kvbk, kvbv, vecs, out_d):
    nc = tc.nc
    from contextlib import ExitStack

    with ExitStack() as ctx:
        # ---- constants / weights: staged on the Activation HWDGE ring so
        # the x chunks own the SP ring ----
        consts = ctx.enter_context(tc.tile_pool(name="consts", bufs=1))
        qwT_sb = consts.tile([128, 4, CQ], BF16)
        nc.scalar.dma_start(out=qwT_sb, in_=qwT)
        qb_sb = consts.tile([128, 2], F32)
        nc.scalar.dma_start(out=qb_sb, in_=qb)
        vecs_sb = consts.tile([128, 4, 18], F32)
        nc.scalar.dma_start(out=vecs_sb, in_=vecs)
        sr2T_sb = consts.tile([128, 4, C], BF16)
        nc.scalar.dma_start(out=sr2T_sb, in_=sr2T)
        kvkT_sb = consts.tile([128, 4, CQ], BF16)
        nc.scalar.dma_start(out=kvkT_sb, in_=kvkT)
        kvvT_sb = consts.tile([128, 4, CQ], BF16)
        nc.scalar.dma_start(out=kvvT_sb, in_=kvvT)
        kvbk_sb = consts.tile([128, 2], F32)
        nc.scalar.dma_start(out=kvbk_sb, in_=kvbk)
        kvbv_sb = consts.tile([128, CQ], F32)
        nc.scalar.dma_start(out=kvbv_sb, in_=kvbv)
        zsmall = consts.tile([128, 1], F32)
        nc.vector.memset(zsmall, 0.0)
        osmall = consts.tile([128, 1], F32)
        nc.vector.memset(osmall, 1.0)
        ones_sb = consts.tile([65, 64], F32R)
        nc.vector.tensor_copy(
            out=ones_sb, in_=osmall[0:65, :].to_broadcast([65, 64])
        )

        persist = ctx.enter_context(tc.tile_pool(name="persist", bufs=1))
        q_sb = persist.tile([128, 2, N], BF16)
        k_loc = persist.tile([128, 2, M], BF16)
        vT_sb = persist.tile([128, 5, HPC, 65], BF16)

        with ExitStack() as pctx:
            xp = pctx.enter_context(tc.tile_pool(name="xp", bufs=1))
            mid = pctx.enter_context(tc.tile_pool(name="mid", bufs=1))
            psB = pctx.enter_context(
                tc.tile_pool(name="psB", bufs=2, space="PSUM")
            )
            psV = pctx.enter_context(
                tc.tile_pool(name="psV", bufs=2, space="PSUM")
            )

            # ---- Phase A: load pre-strided x_ds in per-cc chunks ----
            x_ds = xp.tile([128, 4, HD, WD], BF16)
            for cc in range(4):
                nc.sync.dma_start(out=x_ds[:, cc], in_=xds[:, cc])
            x_flat = x_ds.rearrange("p c h w -> p c (h w)")

            # ---- Phase W: PE warmup during the input load (HAM clock
            # gate releases after ~3.4us of sustained matmul activity) ----
            warm_src = consts.tile([128, 512], F32R)
            nc.vector.tensor_copy(
                out=warm_src, in_=zsmall.to_broadcast([128, 512])
            )
            def pe_warm(nwarm, tag, sink):
                with tc.tile_pool(name=tag, bufs=1, space="PSUM") as psW:
                    wps = psW.tile([128, 512], F32, tag=tag)
                    for i in range(nwarm):
                        nc.tensor.matmul(
                            wps,
                            lhsT=warm_src[:, 0:128],
                            rhs=warm_src,
                            start=(i == 0),
                            stop=(i == nwarm - 1),
                        )
                    # consume so the chain isn't dead
                    nc.vector.tensor_copy(
                        out=warm_src[:, sink:sink + 2], in_=wps[:, 0:2]
                    )

            pe_warm(30, "psW", 0)

            # ---- Phase C: depthwise 2x2 stride-2 pad-1 conv + BN1 + ReLU
            # taps split across Vector and GpSimd ----
            d1 = mid.tile([128, 4, MP], BF16)
            nc.vector.tensor_copy(
                out=d1, in_=zsmall.to_broadcast([128, 4, MP])
            )
            d1s = d1[:, :, 0:M].rearrange("p c (h w) -> p c h w", h=HS)
            xv = x_ds.rearrange("p c (h t) (w u) -> p c h t w u", t=2, u=2)
            for cc in range(4):
                eng = nc.vector
                for ki in (0, 1):
                    ro = slice(1, 25) if ki == 0 else slice(0, 24)
                    for kj in (0, 1):
                        co = slice(1, 25) if kj == 0 else slice(0, 24)
                        src = xv[:, cc, :, 1 - ki, :, 1 - kj]
                        dst = d1s[:, cc, ro, co]
                        eng.scalar_tensor_tensor(
                            out=dst,
                            in0=src,
                            scalar=vecs_sb[:, cc, ki * 2 + kj:ki * 2 + kj + 1],
                            in1=dst,
                            op0=ALU.mult,
                            op1=ALU.add,
                        )
            for cc in range(4):
                nc.scalar.activation(
                    out=d1s[:, cc],
                    in_=d1s[:, cc],
                    func=ACTF.Relu,
                    bias=vecs_sb[:, cc, 5:6],
                    scale=vecs_sb[:, cc, 4:5],
                )

            # ---- Phase B: q projection (256 out-ch, K=512) ----
            for mt in range(2):
                for (n0, nn) in NCH:
                    ps = psB.tile([128, MP], F32, tag="psB")
                    for cc in range(4):
                        nc.tensor.matmul(
                            ps[:, 0:nn],
                            lhsT=qwT_sb[:, cc, mt * 128:(mt + 1) * 128],
                            rhs=x_flat[:, cc, n0:n0 + nn],
                            start=(cc == 0),
                            stop=(cc == 3),
                        )
                    nc.scalar.activation(
                        out=q_sb[:, mt, n0:n0 + nn],
                        in_=ps[:, 0:nn],
                        func=ACTF.Identity,
                        bias=qb_sb[:, mt:mt + 1],
                        scale=1.0,
                    )

            # ---- Phase D: sr2 1x1 conv + BN2 ----
            d1f = d1
            kv1 = mid.tile([128, 4, HS, WS], BF16)
            kv1f = kv1.rearrange("p c h w -> p c (h w)")
            for mt in range(4):
                ps = psB.tile([128, MP], F32, tag="psB")
                for (f0, ff) in MFREE:
                    for cc in range(4):
                        nc.tensor.matmul(
                            ps[:, f0:f0 + ff],
                            lhsT=sr2T_sb[:, cc, mt * 128:(mt + 1) * 128],
                            rhs=d1f[:, cc, f0:f0 + ff],
                            start=(cc == 0),
                            stop=(cc == 3),
                        )
                nc.scalar.activation(
                    out=kv1f[:, mt],
                    in_=ps[:, 0:M],
                    func=ACTF.Identity,
                    bias=vecs_sb[:, mt, 7:8],
                    scale=vecs_sb[:, mt, 6:7],
                )

            # keep the PE's HAM activity window busy across the E-phase
            # DVE wall so F and G start at full clock
            pe_warm(32, "psW2", 2)

            # ---- Phase E: depthwise 3x3 pad-1 conv + lc_b + residual ----
            kv2 = mid.tile([128, 4, MP], BF16)
            nc.vector.tensor_copy(
                out=kv2[:, :, M:MP], in_=zsmall.to_broadcast([128, 4, MP - M])
            )
            kv2s = kv2[:, :, 0:M].rearrange("p c (h w) -> p c h w", h=HS)
            for cc in range(4):
                # center tap: kv2 = (w11 + 1) * kv1 + lc_b  (residual folded)
                nc.scalar.activation(
                    out=kv2s[:, cc],
                    in_=kv1[:, cc],
                    func=ACTF.Identity,
                    bias=vecs_sb[:, cc, 17:18],
                    scale=vecs_sb[:, cc, 12:13],
                )
                eng = nc.vector
                for ki in range(3):
                    for kj in range(3):
                        if ki == 1 and kj == 1:
                            continue
                        di, dj = ki - 1, kj - 1
                        a0, a1 = max(0, -di), 25 - max(0, di)
                        b0, b1 = max(0, -dj), 25 - max(0, dj)
                        src = kv1[:, cc, a0 + di:a1 + di, b0 + dj:b1 + dj]
                        dst = kv2s[:, cc, a0:a1, b0:b1]
                        s = 8 + ki * 3 + kj
                        eng.scalar_tensor_tensor(
                            out=dst,
                            in0=src,
                            scalar=vecs_sb[:, cc, s:s + 1],
                            in1=dst,
                            op0=ALU.mult,
                            op1=ALU.add,
                        )

            # ---- Phase F: k and v projections ----
            kv2f = kv2
            for kt in range(2):
                ps = psB.tile([128, MP], F32, tag="psB")
                for (f0, ff) in MFREE:
                    for cc in range(4):
                        nc.tensor.matmul(
                            ps[:, f0:f0 + ff],
                            lhsT=kvkT_sb[:, cc, kt * 128:(kt + 1) * 128],
                            rhs=kv2f[:, cc, f0:f0 + ff],
                            start=(cc == 0),
                            stop=(cc == 3),
                        )
                nc.scalar.activation(
                    out=k_loc[:, kt],
                    in_=ps[:, 0:M],
                    func=ACTF.Identity,
                    bias=kvbk_sb[:, kt:kt + 1],
                    scale=1.0,
                )

            # v, produced directly transposed: vT[m, d] (+ ones column)
            nc.vector.tensor_copy(
                out=vT_sb[:, :, :, 64], in_=osmall.to_broadcast([128, 5, HPC])
            )
            kvbv_h = kvbv_sb.rearrange("p (h d) -> p h d", h=HPC)
            for mi, (m0, msz) in enumerate(MTS):
                ps = psV.tile([128, CQ], F32, tag="psV")
                for cc in range(4):
                    nc.tensor.matmul(
                        ps[:msz],
                        lhsT=kv2f[:, cc, m0:m0 + msz],
                        rhs=kvvT_sb[:, cc],
                        start=(cc == 0),
                        stop=(cc == 3),
                    )
                nc.vector.tensor_tensor(
                    out=vT_sb[:msz, mi, :, 0:64],
                    in0=ps[:msz].rearrange("p (h d) -> p h d", h=HPC),
                    in1=kvbv_h[:msz],
                    op=ALU.add,
                )

        # ---- Phase G: attention, normalize, W-replicate, store ----
        # Heads are processed in pairs so each store covers 128 contiguous
        # output channels. The x4 column replication is fused into the
        # normalize op; the x4 row replication is four DMAs of the same
        # tile into the f-major output layout, each fully contiguous per
        # partition (128 partitions -> all 16 SDMA engines, ~HBM line
        # rate), alternating HWDGE rings.
        with ExitStack() as gctx:
            oa_pool = gctx.enter_context(
                tc.tile_pool(name="oa", bufs=3, space="PSUM")
            )
            qk_pool = gctx.enter_context(
                tc.tile_pool(name="qk", bufs=5, space="PSUM")
            )
            e_pool = gctx.enter_context(tc.tile_pool(name="es", bufs=6))
            nw_pool = gctx.enter_context(tc.tile_pool(name="nw", bufs=4))
            r_pool = gctx.enter_context(tc.tile_pool(name="rp", bufs=2))

            # The reciprocal / PE-broadcast / normalize of head-chunk j is
            # emitted after head-chunk j+1's matmuls: the tiny `bc`
            # broadcast matmul then reaches the PE with its DVE deps long
            # resolved, so the PE stream stays dense (HAM stays warm).
            pending = []   # (oa, nw, hp, nn)
            ready = {}     # id(nw) -> [norms_done, store_args]
            si = 0

            def emit_norm():
                oa, nw, hp, nn = pending.pop(0)
                rp = r_pool.tile([65, 512], F32, tag="rp")
                nc.vector.reciprocal_approx_fast(
                    out=rp[:, 0:nn], in_=oa[:, 0:nn]
                )
                rt = e_pool.tile([128, 512], F32R, tag="es")
                nc.vector.tensor_copy(
                    out=rt[64:65, 0:nn], in_=rp[64:65, 0:nn]
                )
                bc = qk_pool.tile([64, 512], F32, tag="qk")
                nc.tensor.matmul(
                    bc[:, 0:nn],
                    lhsT=ones_sb[64:65, :],
                    rhs=rt[64:65, 0:nn],
                    start=True,
                    stop=True,
                )
                # TT can read only one PSUM operand: stage 1/rowsum in
                # SBUF via ACT (ScalarE has slack here)
                bcs = e_pool.tile([128, 512], F32R, tag="es")
                nc.vector.tensor_copy(out=bcs[0:64, 0:nn], in_=bc[:, 0:nn])
                # fused normalize + x4 column replication
                nwr = nw[hp:hp + 64].rearrange("p a (w s) -> p (a w) s", s=4)
                src0 = oa[0:64, 0:nn].unsqueeze(2).broadcast_to([64, nn, 4])
                src1 = bcs[0:64, 0:nn].unsqueeze(2).broadcast_to([64, nn, 4])
                # (GpSimd cannot read PSUM: normalize stays on DVE)
                nc.vector.tensor_tensor(
                    out=nwr, in0=src0, in1=src1, op=ALU.mult
                )
                st = ready[id(nw)]
                st[0] += 1
                if st[0] == 2:
                    emit_store(nw, *st[1])

            def emit_store(nw, pp, a0, nr):
                nonlocal si
                # x4 row replication: store the tile into all four
                # f-planes of the f-major output layout
                for f in range(4):
                    eng = nc.sync if (si + f) % 2 == 0 else nc.scalar
                    eng.dma_start(
                        out=out_d[f, pp * 128:(pp + 1) * 128, a0:a0 + nr, :],
                        in_=nw,
                    )
                si += 1

            for pp in range(2):          # head pair: heads (2pp, 2pp+1)
                for (n0, nn) in NCH:
                    nr = nn // WD        # whole 48-wide rows in this chunk
                    a0 = n0 // WD
                    nw = nw_pool.tile([128, nr, W0], F32, tag="nw")
                    ready[id(nw)] = [0, (pp, a0, nr)]
                    for hh in range(2):
                        h = pp * 2 + hh
                        hp = hh * 64
                        oa = oa_pool.tile([65, 512], F32, tag="oa")
                        # all QK matmuls first (exp trails on ScalarE),
                        # then all AV matmuls: keeps the PE stream dense
                        es = []
                        for mi, (m0, msz) in enumerate(MTS):
                            qs = qk_pool.tile([128, 512], F32, tag="qk")
                            nc.tensor.matmul(
                                qs[:msz, 0:nn],
                                lhsT=k_loc[hp:hp + 64, pp, m0:m0 + msz],
                                rhs=q_sb[hp:hp + 64, pp, n0:n0 + nn],
                                start=True,
                                stop=True,
                            )
                            e = e_pool.tile([128, 512], BF16, tag="es")
                            nc.scalar.activation(
                                out=e[:msz, 0:nn],
                                in_=qs[:msz, 0:nn],
                                func=ACTF.Exp,
                                scale=SCALE,
                            )
                            es.append(e)
                        for mi, (m0, msz) in enumerate(MTS):
                            nc.tensor.matmul(
                                oa[:, 0:nn],
                                lhsT=vT_sb[:msz, mi, h, :],
                                rhs=es[mi][:msz, 0:nn],
                                start=(mi == 0),
                                stop=(mi == 4),
                            )
                        pending.append((oa, nw, hp, nn))
                        while len(pending) > 1:
                            emit_norm()
            while pending:
                emit_norm()


_NC_CACHE = None


def _get_nc():
    global _NC_CACHE
    if _NC_CACHE is None:
        _NC_CACHE = _build_nc()
    return _NC_CACHE


def _prep_in_maps(inputs):
    x = np.asarray(inputs["x"], dtype=np.float32)
    q_w = np.asarray(inputs["q_w"], dtype=np.float32)
    q_b = np.asarray(inputs["q_b"], dtype=np.float32)
    kv_w = np.asarray(inputs["kv_w"], dtype=np.float32)
    kv_b = np.asarray(inputs["kv_b"], dtype=np.float32)
    sr1_w = np.asarray(inputs["sr1_w"], dtype=np.float32)
    bn1 = [np.asarray(inputs[f"bn1_{t}"], dtype=np.float32) for t in "gbmv"]
    sr2_w = np.asarray(inputs["sr2_w"], dtype=np.float32)
    bn2 = [np.asarray(inputs[f"bn2_{t}"], dtype=np.float32) for t in "gbmv"]
    lc_w = np.asarray(inputs["lc_w"], dtype=np.float32)
    lc_b = np.asarray(inputs["lc_b"], dtype=np.float32)

    def chan_layout(vec_2d):
        # [C, k] -> [128, 4, k] with channel = cc*128 + p
        k = vec_2d.shape[1]
        return np.ascontiguousarray(
            vec_2d.reshape(4, 128, k).transpose(1, 0, 2)
        )

    s1 = bn1[0] / np.sqrt(bn1[3] + BN_EPS)
    b1 = bn1[1] - bn1[2] * s1
    s2 = bn2[0] / np.sqrt(bn2[3] + BN_EPS)
    b2 = bn2[1] - bn2[2] * s2
    lc = lc_w.reshape(C, 9).copy()
    lc[:, 4] += 1.0  # fold residual into center tap
    vecs = np.zeros((C, 18), np.float32)
    vecs[:, 0:4] = sr1_w.reshape(C, 4)
    vecs[:, 4] = s1
    vecs[:, 5] = b1
    vecs[:, 6] = s2
    vecs[:, 7] = b2
    vecs[:, 8:17] = lc
    vecs[:, 17] = lc_b
    vecs_l = chan_layout(vecs)
    import ml_dtypes

    bf16 = ml_dtypes.bfloat16
    sr2T_l = chan_layout(sr2_w.T.copy()).astype(bf16)

    in_maps = []
    for b in range(4):
        # shard-prep: the strided nearest-downsample view of this batch,
        # laid out [partition, cc, h, w] with channel = cc*128 + p
        xb_ds = np.ascontiguousarray(
            x[b, :, ::4, ::4].reshape(4, 128, HD, WD).transpose(1, 0, 2, 3)
        ).astype(bf16)
        for hg in range(2):
            sl = slice(hg * CQ, (hg + 1) * CQ)
            qwT = chan_layout(q_w[sl].T.copy()).astype(bf16)
            qb_l = np.ascontiguousarray(q_b[sl].reshape(2, 128).T)
            kvkT = chan_layout(kv_w[sl].T.copy()).astype(bf16)
            kvvT = chan_layout(
                kv_w[C + hg * CQ:C + (hg + 1) * CQ].T.copy()
            ).astype(bf16)
            kvbk = np.ascontiguousarray(kv_b[sl].reshape(2, 128).T)
            kvbv = np.ascontiguousarray(
                np.broadcast_to(
                    kv_b[C + hg * CQ:C + (hg + 1) * CQ], (128, CQ)
                ).copy()
            )
            in_maps.append(
                {
                    "xds": xb_ds,
                    "qwT": qwT,
                    "qb": qb_l,
                    "sr2T": sr2T_l,
                    "kvkT": kvkT,
                    "kvvT": kvvT,
                    "kvbk": kvbk,
                    "kvbv": kvbv,
                    "vecs": vecs_l,
                }
            )
    return in_maps


def run(inputs, trace=False, **spmd_kwargs):
    """Run the SPMD kernel; returns (output, BassKernelResults)."""
    nc = _get_nc()
    in_maps = _prep_in_maps(inputs)
    res = run_bass_kernel_spmd(
        nc, in_maps, core_ids=list(range(NCORES)), trace=trace, **spmd_kwargs
    )
    out = np.empty((4, C, H0, W0), np.float32)
    i = 0
    for b in range(4):
        for hg in range(2):
            # device layout [f, c, a, w] -> slab [c, 4a+f, w]
            dev = res.results[i]["out"]
            out[b, hg * CQ:(hg + 1) * CQ] = (
                dev.transpose(1, 2, 0, 3).reshape(CQ, H0, W0)
            )
            i += 1
    return out, res


def kernel(**inputs):
    out, _ = run(inputs, trace=False)
    return out
